# revision 9
# baseline (speedup 1.0000x reference)
"""LogNeuralJastrowSlater — Trainium2 Bass kernel (8-core data-parallel).

reference:
  J   = sum(tanh(n @ W + b), -1)
  A   = M[sorted nonzero positions of n]          (64x64 per sample)
  out = slogdet(A) as complex log-det + J

Single-pass scheme (samples ride SBUF partitions, 128 per tile). The
determinant core is unpivoted Crout LU (n^3/3 MACs) in fp32, split
across the DVE and GPSIMD(Pool) engines; numerically risky samples
(~10%, detected via per-sample growth/cancellation metrics) are
recomputed on the host with the same fp32 LAPACK slogdet the reference
uses.

  per step k: merged row+col dot products. The DVE takes the row dots
  plus a slice of the col dots (tensor_tensor mult + tensor_reduce);
  the Pool engine takes the rest via scalar_tensor_tensor mult + its
  own tensor_reduce. Work is split ~57/43 to balance the two engines'
  throughput. Pivots land on the diagonal; per-sample risk metrics
  (min|pivot|, max|entry|, rmax2, csum2) ship with the result and the
  host flags the union.

  J via PE matmul + ACT tanh with fused accumulate (off the DVE path).
  idx via 8 rounds of DVE max8/max_index over a keyed copy of n; A
  gathered with 64 indirect-DMA row gathers of M per tile.

Sharding: pure data parallel over the batch dim; 8 cores x 4096 samples.
"""

import numpy as np

import concourse.bass as bass
import concourse.bacc as bacc
import concourse.mybir as mybir
import concourse.tile as tile
from concourse.bass_utils import run_bass_kernel_spmd
from concourse.masks import make_identity

P = 128
B, N_ORB, N_F, HID = 32768, 256, 64, 128
N_CORES = 8
N_TILES = B // N_CORES // P          # 32 sample-tiles per core
G = 4                                # tiles per LU pass (SBUF-limited)
F32 = mybir.dt.float32
U32 = mybir.dt.uint32
Alu = mybir.AluOpType
Act = mybir.ActivationFunctionType

# host-side flagging thresholds (tuned on the reference input distribution;
# max unflagged |err| ~0.30 at ~10% flag rate)
FLAG_RMAX2 = 3.0e5
FLAG_MINPIV = 1.1e-3
FLAG_AMAX = 1.9e3
FLAG_CSUM2 = 8.0e6

_cached_lu = None

A_BUFS = 2   # double-buffer A: overlap gather/idx/J with LU
# Pool share of the product (multiply) rows.  The DVE (1.0417 ns/elem)
# does the remaining products plus ALL segmented reduces; the Pool does
# tensor_tensor multiplies at 1.984 ns/elem.  Balance:
# 1.0417*(2-q)*W == 1.984*q*W  ->  q = 0.688.
POOL_SHARE = 0.688


def _emit_prep(nc, tc, consts, small, ps, A, Jg, gi, T, n_d, M_d):
    """Per-tile front-end: load n, compute J (PE+ACT), occupied idx (8
    rounds of DVE max8/max_index), gather A rows (indirect DMA).  The
    gather ORDER is set by the wkey weights input: (256-o) gives
    ascending orbital order."""
    ident, W_sb, wkey_sb, ones1, b_row = consts
    n_t = small.tile([P, N_ORB], F32, tag="n_t")
    nc.sync.dma_start(n_t[:], n_d[T * P:(T + 1) * P, :])

    # J = sum tanh(n @ W + b)
    ps_tr = ps.tile([P, P], F32, tag="ps_tr")
    nT = small.tile([P, 2, P], F32, tag="nT")
    for c in range(2):
        nc.tensor.transpose(ps_tr[:], n_t[:, c * P:(c + 1) * P], ident[:])
        nc.scalar.copy(nT[:, c, :], ps_tr[:])
    ps_J = ps.tile([P, HID], F32, tag="ps_J")
    for c in range(2):
        nc.tensor.matmul(ps_J[:], lhsT=nT[:, c, :], rhs=W_sb[:, c, :],
                         start=(c == 0), stop=False)
    nc.tensor.matmul(ps_J[:], lhsT=ones1[:], rhs=b_row[:],
                     start=False, stop=True)
    tanh_dump = small.tile([P, HID], F32, tag="tanh_dump")
    nc.scalar.activation(tanh_dump[:], ps_J[:], Act.Tanh,
                         accum_out=Jg[:, gi:gi + 1])

    # idx of the 64 ones, in wkey order
    keyA = small.tile([P, N_ORB], F32, tag="keyA")
    keyB = small.tile([P, N_ORB], F32, tag="keyB")
    nc.vector.tensor_tensor(keyA[:], n_t[:], wkey_sb[:], Alu.mult)
    idxb = small.tile([P, N_F], U32, tag="idxb")
    mx8 = small.tile([P, 8], F32, tag="mx8")
    cur, oth = keyA, keyB
    for r8 in range(8):
        nc.vector.max(mx8[:], cur[:])
        nc.vector.max_index(idxb[:, r8 * 8:(r8 + 1) * 8], mx8[:], cur[:])
        if r8 < 7:
            nc.vector.match_replace(oth[:], mx8[:], cur[:], 0.0)
            cur, oth = oth, cur

    # gather A rows from M
    for r in range(N_F):
        nc.gpsimd.indirect_dma_start(
            out=A[:, gi, r, :], out_offset=None, in_=M_d[:],
            in_offset=bass.IndirectOffsetOnAxis(ap=idxb[:, r:r + 1], axis=0))


def _build_lu_kernel(n_tiles: int, g_sz: int, metrics: bool = True):
    """fp32 unpivoted Crout LU over all samples."""
    S = n_tiles * P
    nc = bacc.Bacc(trn_type="TRN2", target_bir_lowering=False, debug=False)
    n_d = nc.dram_tensor("n_shard", [S, N_ORB], F32, kind="ExternalInput").ap()
    M_d = nc.dram_tensor("Mmat", [N_ORB, N_F], F32, kind="ExternalInput").ap()
    W_d = nc.dram_tensor("Wmat", [N_ORB, HID], F32, kind="ExternalInput").ap()
    b_d = nc.dram_tensor("bvec", [P, HID], F32, kind="ExternalInput").ap()
    wkey_d = nc.dram_tensor("wkey", [P, N_ORB], F32, kind="ExternalInput").ap()
    n_out = 6 if metrics else 2
    out_d = nc.dram_tensor("out6" if metrics else "out2", [S, n_out], F32,
                           kind="ExternalOutput").ap()

    sizes = [g_sz] * (n_tiles // g_sz)
    if n_tiles % g_sz:
        sizes.append(n_tiles % g_sz)

    with tile.TileContext(nc) as tc:
        with tc.tile_pool(name="consts", bufs=1) as consts_p, \
             tc.tile_pool(name="Apool", bufs=A_BUFS) as Apool, \
             tc.tile_pool(name="prod", bufs=1) as prodp, \
             tc.tile_pool(name="small", bufs=1) as small, \
             tc.tile_pool(name="lup", bufs=1) as lup, \
             tc.tile_pool(name="ps", bufs=2, space="PSUM") as ps:

            ident = consts_p.tile([P, P], F32)
            make_identity(nc, ident[:])
            W_sb = consts_p.tile([P, 2, HID], F32)
            nc.sync.dma_start(W_sb[:], W_d[:].rearrange("(c p) h -> p c h", p=P))
            wkey_sb = consts_p.tile([P, N_ORB], F32)
            nc.sync.dma_start(wkey_sb[:], wkey_d[:])
            ones1 = consts_p.tile([1, P], F32)
            nc.vector.memset(ones1[:], 1.0)
            b_row = consts_p.tile([1, HID], F32)
            nc.sync.dma_start(b_row[:], b_d[0:1, :])
            consts = (ident, W_sb, wkey_sb, ones1, b_row)

            g_lo = 0
            for g_n in sizes:
                A = Apool.tile([P, g_sz, N_F, N_F], F32, tag="A", name="A")
                Jg = lup.tile([P, g_sz], F32, tag="Jg", name="Jg")

                for gi in range(g_n):
                    _emit_prep(nc, tc, consts, small, ps, A, Jg, gi,
                               g_lo + gi, n_d, M_d)

                # ---- in-place unpivoted Crout LU, g_n tiles per instruction
                Pt = prodp.tile([P, g_sz, 2048], F32, tag="Pt")
                D = lup.tile([P, g_sz, 2 * N_F], F32, tag="D")
                rv = lup.tile([P, g_sz], F32, tag="rv")
                t1 = lup.tile([P, g_sz, N_F - 1], F32, tag="t1")
                if metrics:
                    r1 = lup.tile([P, g_sz], F32, tag="r1")
                    c1 = lup.tile([P, g_sz], F32, tag="c1")
                    rmax2 = lup.tile([P, g_sz], F32, tag="rmax2")
                    csum2 = lup.tile([P, g_sz], F32, tag="csum2")
                    nc.vector.memset(rmax2[:, :g_n], 0.0)
                    nc.vector.memset(csum2[:, :g_n], 0.0)

                Ag = A[:, :g_n]
                for k in range(N_F):
                    m = N_F - k
                    if k > 0:
                        # merged row+col products into Pt[0:(2m-1)*k].
                        # The Pool engine (scalar_tensor_tensor mult at
                        # 1.389 ns/elem) takes POOL_SHARE of the product
                        # rows; the DVE does the rest of the products plus
                        # ALL the segmented reduces (1.0417 ns/elem).
                        nrow_all = 2 * m - 1
                        pool_rows = 0
                        if k >= 2:
                            pool_rows = min(nrow_all - 1,
                                            round(POOL_SHARE * nrow_all))
                        # DVE always keeps at least row 0 (the pivot row dot)
                        dve_r = m - max(0, pool_rows - (m - 1))  # DVE row-rows
                        # row-products: rows 0..m (outputs u row k)
                        in0r = Ag[:, :, k, 0:k]
                        in1r = Ag[:, :, 0:k, k:].rearrange("p g t j -> p g j t")
                        if dve_r > 0:
                            prow = Pt[:, :g_n, 0:dve_r * k].rearrange(
                                "p g (i t) -> p g i t", t=k)
                            nc.vector.tensor_tensor(
                                prow, in0r.unsqueeze(2).broadcast_to(
                                    [P, g_n, dve_r, k]),
                                in1r[:, :, 0:dve_r], Alu.mult)
                        if dve_r < m:
                            nrp = m - dve_r
                            prow2 = Pt[:, :g_n, dve_r * k:m * k].rearrange(
                                "p g (i t) -> p g i t", t=k)
                            nc.gpsimd.tensor_tensor(
                                prow2, in0r.unsqueeze(2).broadcast_to(
                                    [P, g_n, nrp, k]),
                                in1r[:, :, dve_r:m], Alu.mult)
                        if m > 1:
                            # col-products: rows m..2m-1 (outputs l col k)
                            pool_c = min(m - 1, pool_rows)
                            dve_c = m - 1 - pool_c
                            in1c = Ag[:, :, 0:k, k]
                            if dve_c > 0:
                                pcol = Pt[:, :g_n,
                                          m * k:(m + dve_c) * k].rearrange(
                                    "p g (i t) -> p g i t", t=k)
                                nc.vector.tensor_tensor(
                                    pcol, Ag[:, :, k + 1:k + 1 + dve_c, 0:k],
                                    in1c.unsqueeze(2).broadcast_to(
                                        [P, g_n, dve_c, k]), Alu.mult)
                            if pool_c > 0:
                                pcolp = Pt[:, :g_n,
                                           (m + dve_c) * k:nrow_all * k
                                           ].rearrange(
                                    "p g (i t) -> p g i t", t=k)
                                nc.gpsimd.tensor_tensor(
                                    pcolp,
                                    Ag[:, :, k + 1 + dve_c:, 0:k],
                                    in1c.unsqueeze(2).broadcast_to(
                                        [P, g_n, pool_c, k]), Alu.mult)
                        # one segmented reduce over every product row (DVE).
                        # Pt rows: [0..m) = u-row dots, [m..2m-1) = l-col
                        # dots (dve slice first, then pool slice)
                        pall = Pt[:, :g_n, 0:nrow_all * k].rearrange(
                            "p g (i t) -> p g i t", t=k)
                        nc.vector.tensor_reduce(D[:, :g_n, 0:nrow_all], pall,
                                                mybir.AxisListType.X, Alu.add)
                        # u row k (in place, includes pivot at A[k,k])
                        nc.vector.tensor_tensor(Ag[:, :, k, k:], Ag[:, :, k, k:],
                                                D[:, :g_n, 0:m], Alu.subtract)
                    nc.vector.reciprocal(rv[:, :g_n], Ag[:, :, k, k])
                    if k > 0 and metrics:
                        # cancellation metric: rmax2 = max_k (dot_piv/u_kk)^2
                        nc.vector.tensor_tensor(r1[:, :g_n], D[:, :g_n, 0],
                                                rv[:, :g_n], Alu.mult)
                        nc.vector.tensor_tensor(r1[:, :g_n], r1[:, :g_n],
                                                r1[:, :g_n], Alu.mult)
                        nc.vector.tensor_tensor(rmax2[:, :g_n], rmax2[:, :g_n],
                                                r1[:, :g_n], Alu.max)
                        if m > 1:
                            # csum2 += (max_i |cdot_i| / u_kk)^2
                            nc.vector.tensor_reduce(
                                c1[:, :g_n], D[:, :g_n, m:2 * m - 1],
                                mybir.AxisListType.X, Alu.max,
                                apply_absolute_value=True)
                            nc.vector.tensor_tensor(c1[:, :g_n], c1[:, :g_n],
                                                    rv[:, :g_n], Alu.mult)
                            nc.vector.tensor_tensor(c1[:, :g_n], c1[:, :g_n],
                                                    c1[:, :g_n], Alu.mult)
                            nc.vector.tensor_tensor(csum2[:, :g_n],
                                                    csum2[:, :g_n],
                                                    c1[:, :g_n], Alu.add)
                    if m > 1:
                        rvb = rv[:, :g_n].unsqueeze(2).broadcast_to(
                            [P, g_n, m - 1])
                        if k > 0:
                            # l column: (a - cdot) * (1/u_kk), in place
                            nc.vector.scalar_tensor_tensor(
                                t1[:, :g_n, 0:m - 1], D[:, :g_n, m:2 * m - 1],
                                -1.0, Ag[:, :, k + 1:, k], Alu.mult, Alu.add)
                            nc.vector.tensor_tensor(Ag[:, :, k + 1:, k],
                                                    t1[:, :g_n, 0:m - 1], rvb,
                                                    Alu.mult)
                        else:
                            nc.vector.tensor_tensor(Ag[:, :, 1:, 0],
                                                    Ag[:, :, 1:, 0], rvb,
                                                    Alu.mult)

                # ---- flags + output
                Aflat = Ag.rearrange("p g i j -> p g (i j)")
                diag = Aflat[:, :, 0:N_F * N_F:N_F + 1]
                if metrics:
                    amax = lup.tile([P, g_sz], F32, tag="amax")
                    nc.vector.tensor_reduce(amax[:, :g_n], Aflat,
                                            mybir.AxisListType.X, Alu.max,
                                            apply_absolute_value=True)
                    minpiv = lup.tile([P, g_sz], F32, tag="minpiv")
                    nc.vector.tensor_reduce(minpiv[:, :g_n], diag,
                                            mybir.AxisListType.X, Alu.min,
                                            apply_absolute_value=True)
                absd = lup.tile([P, g_sz, N_F], F32, tag="absd")
                nc.scalar.activation(absd[:, :g_n], diag, Act.Abs)
                lnd = lup.tile([P, g_sz, N_F], F32, tag="lnd")
                nc.scalar.activation(lnd[:, :g_n], absd[:, :g_n], Act.Ln)
                logabs = lup.tile([P, g_sz], F32, tag="logabs")
                nc.vector.tensor_reduce(logabs[:, :g_n], lnd[:, :g_n],
                                        mybir.AxisListType.X, Alu.add)
                sg = lup.tile([P, g_sz, N_F], F32, tag="sg")
                nc.vector.tensor_scalar(out=sg[:, :g_n], in0=diag,
                                        scalar1=0.0, scalar2=-2.0,
                                        op0=Alu.is_lt, op1=Alu.mult)
                nc.vector.tensor_scalar_add(sg[:, :g_n], sg[:, :g_n], 1.0)
                prodsg = lup.tile([P, g_sz], F32, tag="prodsg")
                nc.vector.tensor_reduce(prodsg[:, :g_n], sg[:, :g_n],
                                        mybir.AxisListType.X, Alu.mult)
                out_t = lup.tile([P, g_sz, n_out], F32, tag="out_t")
                nc.vector.tensor_tensor(out_t[:, :g_n, 0], logabs[:, :g_n],
                                        Jg[:, :g_n], Alu.add)
                nc.vector.tensor_scalar(out=out_t[:, :g_n, 1],
                                        in0=prodsg[:, :g_n],
                                        scalar1=0.0, scalar2=float(np.pi),
                                        op0=Alu.is_lt, op1=Alu.mult)
                if metrics:
                    nc.vector.tensor_copy(out_t[:, :g_n, 2], minpiv[:, :g_n])
                    nc.vector.tensor_copy(out_t[:, :g_n, 3], amax[:, :g_n])
                    nc.vector.tensor_copy(out_t[:, :g_n, 4], rmax2[:, :g_n])
                    nc.vector.tensor_copy(out_t[:, :g_n, 5], csum2[:, :g_n])
                od = out_d[g_lo * P:(g_lo + g_n) * P, :]
                od_pgc = bass.AP(od.tensor, od.offset,
                                 [[n_out, P], [n_out * P, g_n], [1, n_out]])
                nc.sync.dma_start(od_pgc, out_t[:, :g_n])
                g_lo += g_n

    nc.compile()
    return nc


def _get_lu():
    global _cached_lu
    if _cached_lu is None:
        _cached_lu = _build_lu_kernel(N_TILES, G)
    return _cached_lu


def _shared_inputs(M, W, b):
    return {
        "Mmat": np.ascontiguousarray(M), "Wmat": np.ascontiguousarray(W),
        "bvec": np.ascontiguousarray(b[None, :].repeat(P, 0)),
        "wkey": np.ascontiguousarray(
            (N_ORB - np.arange(N_ORB, dtype=np.float32))[None, :].repeat(P, 0)),
    }


def kernel(n, M, W, b, _trace=False):
    n = np.ascontiguousarray(np.asarray(n, dtype=np.float32))
    M = np.ascontiguousarray(np.asarray(M, dtype=np.float32))
    W = np.ascontiguousarray(np.asarray(W, dtype=np.float32))
    b = np.asarray(b, dtype=np.float32)
    assert n.shape == (B, N_ORB) and M.shape == (N_ORB, N_F)

    shared = _shared_inputs(M, W, b)
    S = B // N_CORES

    # ---- pass 1: fp32 unpivoted LU over everything
    nc1 = _get_lu()
    in_maps = [dict(shared, n_shard=np.ascontiguousarray(n[c * S:(c + 1) * S]))
               for c in range(N_CORES)]
    res = run_bass_kernel_spmd(nc1, in_maps, core_ids=list(range(N_CORES)),
                               trace=_trace)
    out = np.empty((B,), np.complex64)
    minpiv = np.empty(B, np.float32)
    amax = np.empty(B, np.float32)
    rmax2 = np.empty(B, np.float32)
    csum2 = np.empty(B, np.float32)
    for c in range(N_CORES):
        o6 = res.results[c]["out6"]
        out[c * S:(c + 1) * S] = o6[:, 0] + 1j * o6[:, 1]
        minpiv[c * S:(c + 1) * S] = o6[:, 2]
        amax[c * S:(c + 1) * S] = o6[:, 3]
        rmax2[c * S:(c + 1) * S] = o6[:, 4]
        csum2[c * S:(c + 1) * S] = o6[:, 5]

    # ---- host-side flagging of numerically-risky samples
    with np.errstate(invalid="ignore"):
        bad = (~np.isfinite(out.real)) | (~np.isfinite(rmax2)) \
            | (~np.isfinite(csum2)) | (rmax2 > FLAG_RMAX2) \
            | (minpiv < FLAG_MINPIV) | (amax > FLAG_AMAX) \
            | (csum2 > FLAG_CSUM2)
    flagged = np.nonzero(bad)[0]
    kernel._last_flagged = len(flagged)

    # ---- pass 2: repair flagged samples on host with the same fp32
    # LAPACK slogdet the reference uses (pivoted, so numerically robust).
    if len(flagged) > 0:
        nn = n[flagged]
        idxs = np.argsort(-nn, axis=1, kind="stable")[:, :N_F]
        idxs.sort(axis=1)
        sign, logabs = np.linalg.slogdet(M[idxs])
        Jsel = np.tanh(nn @ W + b[None, :]).sum(axis=1)
        out[flagged] = (logabs + Jsel).astype(np.float32) \
            + 1j * np.where(sign < 0, np.float32(np.pi), np.float32(0.0))

    if _trace:
        kernel._last_results = res
    return out


# revision 13
# speedup vs baseline: 1.2644x; 1.2644x over previous
"""LogNeuralJastrowSlater — Trainium2 Bass kernel (8-core data-parallel).

reference:
  J   = sum(tanh(n @ W + b), -1)
  A   = M[sorted nonzero positions of n]          (64x64 per sample)
  out = slogdet(A) as complex log-det + J

Single-pass scheme (samples ride SBUF partitions, 128 per tile). The
determinant core is unpivoted Crout LU (n^3/3 MACs) in fp32, split
across the DVE and GPSIMD(Pool) engines; numerically risky samples
(~10%, detected via per-sample growth/cancellation metrics) are
recomputed on the host with the same fp32 LAPACK slogdet the reference
uses.

  per step k: merged row+col dot products. The DVE takes the row dots
  plus a slice of the col dots (tensor_tensor mult + tensor_reduce);
  the Pool engine takes the rest via scalar_tensor_tensor mult + its
  own tensor_reduce. Work is split ~57/43 to balance the two engines'
  throughput. Pivots land on the diagonal; per-sample risk metrics
  (min|pivot|, max|entry|, rmax2, csum2) ship with the result and the
  host flags the union.

  J via PE matmul + ACT tanh with fused accumulate (off the DVE path).
  idx via 8 rounds of DVE max8/max_index over a keyed copy of n; A
  gathered with 64 indirect-DMA row gathers of M per tile.

Sharding: pure data parallel over the batch dim; 8 cores x 4096 samples.
"""

import numpy as np

import concourse.bass as bass
import concourse.bacc as bacc
import concourse.mybir as mybir
import concourse.tile as tile
from concourse.bass_utils import run_bass_kernel_spmd
from concourse.masks import make_identity

P = 128
B, N_ORB, N_F, HID = 32768, 256, 64, 128
N_CORES = 8
N_TILES = B // N_CORES // P          # 32 sample-tiles per core
G = 4                                # tiles per LU pass (SBUF-limited)
F32 = mybir.dt.float32
U32 = mybir.dt.uint32
Alu = mybir.AluOpType
Act = mybir.ActivationFunctionType

# host-side flagging thresholds (tuned on the reference input distribution)
FLAG_RMAX2 = 3.0e5
FLAG_MINPIV = 1.1e-3
FLAG_AMAX = 1.9e3
FLAG_LMAX = 20.0     # max_k max_i |l_ik| — elimination growth

_cached_lu = None

A_BUFS = 2   # double-buffer A: overlap gather/idx/J with LU
# Pool share of the dot-product rows.  Both engines run an INDEPENDENT
# end-to-end chain per step (products + reduce/tree + their slice of the
# l-column update) so neither stalls on the other mid-step.  DVE runs
# 1.0417 ns/elem, Pool tensor_tensor runs 1.984 ns/elem; balance
# 1.0417*(2m-1-p) == 1.984*p  ->  p = 0.345*(2m-1).
POOL_SHARE = 0.345
POOL_MIN_K = 6   # below this dot width the pool tree overhead dominates


def _emit_prep(nc, tc, consts, small, ps, A, Jg, gi, T, n_d, M_d):
    """Per-tile front-end: load n, compute J (PE+ACT), occupied idx (8
    rounds of DVE max8/max_index), gather A rows (indirect DMA).  The
    gather ORDER is set by the wkey weights input: (256-o) gives
    ascending orbital order."""
    ident, W_sb, wkey_sb, ones1, b_row = consts
    n_t = small.tile([P, N_ORB], F32, tag="n_t")
    nc.sync.dma_start(n_t[:], n_d[T * P:(T + 1) * P, :])

    # J = sum tanh(n @ W + b)
    ps_tr = ps.tile([P, P], F32, tag="ps_tr")
    nT = small.tile([P, 2, P], F32, tag="nT")
    for c in range(2):
        nc.tensor.transpose(ps_tr[:], n_t[:, c * P:(c + 1) * P], ident[:])
        nc.scalar.copy(nT[:, c, :], ps_tr[:])
    ps_J = ps.tile([P, HID], F32, tag="ps_J")
    for c in range(2):
        nc.tensor.matmul(ps_J[:], lhsT=nT[:, c, :], rhs=W_sb[:, c, :],
                         start=(c == 0), stop=False)
    nc.tensor.matmul(ps_J[:], lhsT=ones1[:], rhs=b_row[:],
                     start=False, stop=True)
    tanh_dump = small.tile([P, HID], F32, tag="tanh_dump")
    nc.scalar.activation(tanh_dump[:], ps_J[:], Act.Tanh,
                         accum_out=Jg[:, gi:gi + 1])

    # idx of the 64 ones, in wkey order
    keyA = small.tile([P, N_ORB], F32, tag="keyA")
    keyB = small.tile([P, N_ORB], F32, tag="keyB")
    nc.vector.tensor_tensor(keyA[:], n_t[:], wkey_sb[:], Alu.mult)
    idxb = small.tile([P, N_F], U32, tag="idxb")
    mx8 = small.tile([P, 8], F32, tag="mx8")
    cur, oth = keyA, keyB
    for r8 in range(8):
        nc.vector.max(mx8[:], cur[:])
        nc.vector.max_index(idxb[:, r8 * 8:(r8 + 1) * 8], mx8[:], cur[:])
        if r8 < 7:
            nc.vector.match_replace(oth[:], mx8[:], cur[:], 0.0)
            cur, oth = oth, cur

    # gather A rows from M
    for r in range(N_F):
        nc.gpsimd.indirect_dma_start(
            out=A[:, gi, r, :], out_offset=None, in_=M_d[:],
            in_offset=bass.IndirectOffsetOnAxis(ap=idxb[:, r:r + 1], axis=0))


def _build_lu_kernel(n_tiles: int, g_sz: int, metrics: bool = True):
    """fp32 unpivoted Crout LU over all samples."""
    S = n_tiles * P
    nc = bacc.Bacc(trn_type="TRN2", target_bir_lowering=False, debug=False)
    n_d = nc.dram_tensor("n_shard", [S, N_ORB], F32, kind="ExternalInput").ap()
    M_d = nc.dram_tensor("Mmat", [N_ORB, N_F], F32, kind="ExternalInput").ap()
    W_d = nc.dram_tensor("Wmat", [N_ORB, HID], F32, kind="ExternalInput").ap()
    b_d = nc.dram_tensor("bvec", [P, HID], F32, kind="ExternalInput").ap()
    wkey_d = nc.dram_tensor("wkey", [P, N_ORB], F32, kind="ExternalInput").ap()
    n_out = 6 if metrics else 2
    out_d = nc.dram_tensor("out6" if metrics else "out2", [S, n_out], F32,
                           kind="ExternalOutput").ap()

    sizes = [g_sz] * (n_tiles // g_sz)
    if n_tiles % g_sz:
        sizes.append(n_tiles % g_sz)

    with tile.TileContext(nc) as tc:
        with tc.tile_pool(name="consts", bufs=1) as consts_p, \
             tc.tile_pool(name="Apool", bufs=A_BUFS) as Apool, \
             tc.tile_pool(name="prod", bufs=1) as prodp, \
             tc.tile_pool(name="small", bufs=1) as small, \
             tc.tile_pool(name="lup", bufs=1) as lup, \
             tc.tile_pool(name="ps", bufs=2, space="PSUM") as ps:

            ident = consts_p.tile([P, P], F32)
            make_identity(nc, ident[:])
            W_sb = consts_p.tile([P, 2, HID], F32)
            nc.sync.dma_start(W_sb[:], W_d[:].rearrange("(c p) h -> p c h", p=P))
            wkey_sb = consts_p.tile([P, N_ORB], F32)
            nc.sync.dma_start(wkey_sb[:], wkey_d[:])
            ones1 = consts_p.tile([1, P], F32)
            nc.vector.memset(ones1[:], 1.0)
            b_row = consts_p.tile([1, HID], F32)
            nc.sync.dma_start(b_row[:], b_d[0:1, :])
            consts = (ident, W_sb, wkey_sb, ones1, b_row)

            g_lo = 0
            for g_n in sizes:
                A = Apool.tile([P, g_sz, N_F, N_F], F32, tag="A", name="A")
                Jg = lup.tile([P, g_sz], F32, tag="Jg", name="Jg")

                for gi in range(g_n):
                    _emit_prep(nc, tc, consts, small, ps, A, Jg, gi,
                               g_lo + gi, n_d, M_d)

                # ---- in-place unpivoted Crout LU, g_n tiles per instruction
                Pt = prodp.tile([P, g_sz, 2048], F32, tag="Pt")
                D = lup.tile([P, g_sz, 2 * N_F], F32, tag="D")
                rv = lup.tile([P, g_sz], F32, tag="rv")
                t1 = lup.tile([P, g_sz, N_F - 1], F32, tag="t1")
                if metrics:
                    r1 = lup.tile([P, g_sz], F32, tag="r1")
                    c1 = lup.tile([P, g_sz], F32, tag="c1")
                    rmax2 = lup.tile([P, g_sz], F32, tag="rmax2")
                    csum2 = lup.tile([P, g_sz], F32, tag="csum2")
                    nc.vector.memset(rmax2[:, :g_n], 0.0)
                    nc.vector.memset(csum2[:, :g_n], 0.0)

                Ag = A[:, :g_n]
                for k in range(N_F):
                    m = N_F - k
                    # col-row split: pool takes pool_c of the m-1 l-column
                    # rows end-to-end; DVE takes the rest plus the u row.
                    pool_c = 0
                    if k >= POOL_MIN_K and m > 2:
                        pool_c = min(m - 1, round(POOL_SHARE * (2 * m - 1)))
                    dve_c = m - 1 - pool_c
                    if k > 0:
                        # DVE: row products + its col slice into Pt
                        in0r = Ag[:, :, k, 0:k]
                        in1r = Ag[:, :, 0:k, k:].rearrange("p g t j -> p g j t")
                        prow = Pt[:, :g_n, 0:m * k].rearrange(
                            "p g (i t) -> p g i t", t=k)
                        nc.vector.tensor_tensor(
                            prow, in0r.unsqueeze(2).broadcast_to(
                                [P, g_n, m, k]), in1r, Alu.mult)
                        in1c = Ag[:, :, 0:k, k]
                        if dve_c > 0:
                            pcol = Pt[:, :g_n,
                                      m * k:(m + dve_c) * k].rearrange(
                                "p g (i t) -> p g i t", t=k)
                            nc.vector.tensor_tensor(
                                pcol, Ag[:, :, k + 1:k + 1 + dve_c, 0:k],
                                in1c.unsqueeze(2).broadcast_to(
                                    [P, g_n, dve_c, k]), Alu.mult)
                        if pool_c > 0:
                            # Pool chain: products + halving-tree reduce,
                            # independent of the DVE's step work
                            base = (m + dve_c) * k
                            pcolp = Pt[:, :g_n,
                                       base:(2 * m - 1) * k].rearrange(
                                "p g (i t) -> p g i t", t=k)
                            nc.gpsimd.tensor_tensor(
                                pcolp, Ag[:, :, k + 1 + dve_c:, 0:k],
                                in1c.unsqueeze(2).broadcast_to(
                                    [P, g_n, pool_c, k]), Alu.mult)
                            L = k

                            def pv(lo, cnt):
                                v = Pt[:, :g_n, base:base + pool_c * k]
                                v = v.rearrange("p g (i t) -> p g i t", t=k)
                                return v[:, :, :, lo:lo + cnt]
                            while L > 1:
                                if L % 2:
                                    nc.gpsimd.tensor_tensor(
                                        pv(L - 2, 1), pv(L - 2, 1),
                                        pv(L - 1, 1), Alu.add)
                                    L -= 1
                                h = L // 2
                                nc.gpsimd.tensor_tensor(
                                    pv(0, h), pv(0, h), pv(h, h), Alu.add)
                                L = h
                        # DVE reduce over its own rows only
                        nr = m + dve_c
                        pall = Pt[:, :g_n, 0:nr * k].rearrange(
                            "p g (i t) -> p g i t", t=k)
                        nc.vector.tensor_reduce(D[:, :g_n, 0:nr], pall,
                                                mybir.AxisListType.X, Alu.add)
                        # u row k (in place, includes pivot at A[k,k])
                        nc.vector.tensor_tensor(Ag[:, :, k, k:], Ag[:, :, k, k:],
                                                D[:, :g_n, 0:m], Alu.subtract)
                    nc.vector.reciprocal(rv[:, :g_n], Ag[:, :, k, k])
                    if k > 0 and metrics:
                        # cancellation metric: rmax2 = max_k (dot_piv/u_kk)^2
                        nc.vector.tensor_tensor(r1[:, :g_n], D[:, :g_n, 0],
                                                rv[:, :g_n], Alu.mult)
                        nc.vector.tensor_tensor(r1[:, :g_n], r1[:, :g_n],
                                                r1[:, :g_n], Alu.mult)
                        nc.vector.tensor_tensor(rmax2[:, :g_n], rmax2[:, :g_n],
                                                r1[:, :g_n], Alu.max)
                    if m > 1:
                        if k > 0:
                            # DVE slice of l col: (a - cdot) * (1/u_kk)
                            if dve_c > 0:
                                rvb = rv[:, :g_n].unsqueeze(2).broadcast_to(
                                    [P, g_n, dve_c])
                                nc.vector.scalar_tensor_tensor(
                                    t1[:, :g_n, 0:dve_c],
                                    D[:, :g_n, m:m + dve_c],
                                    -1.0, Ag[:, :, k + 1:k + 1 + dve_c, k],
                                    Alu.mult, Alu.add)
                                nc.vector.tensor_tensor(
                                    Ag[:, :, k + 1:k + 1 + dve_c, k],
                                    t1[:, :g_n, 0:dve_c], rvb, Alu.mult)
                            if pool_c > 0:
                                # Pool applies its own slice of the l col
                                rvbp = rv[:, :g_n].unsqueeze(2).broadcast_to(
                                    [P, g_n, pool_c])
                                Acolp = Ag[:, :, k + 1 + dve_c:, k]
                                nc.gpsimd.tensor_tensor(
                                    Acolp, Acolp, pv(0, 1).squeeze(3),
                                    Alu.subtract)
                                nc.gpsimd.tensor_tensor(
                                    Acolp, Acolp, rvbp, Alu.mult)
                        else:
                            rvb = rv[:, :g_n].unsqueeze(2).broadcast_to(
                                [P, g_n, m - 1])
                            nc.vector.tensor_tensor(Ag[:, :, 1:, 0],
                                                    Ag[:, :, 1:, 0], rvb,
                                                    Alu.mult)
                        if metrics:
                            # growth metric: lmax = max_k max_i |l_ik|
                            nc.vector.tensor_reduce(
                                c1[:, :g_n], Ag[:, :, k + 1:, k],
                                mybir.AxisListType.X, Alu.max,
                                apply_absolute_value=True)
                            nc.vector.tensor_tensor(csum2[:, :g_n],
                                                    csum2[:, :g_n],
                                                    c1[:, :g_n], Alu.max)

                # ---- flags + output
                Aflat = Ag.rearrange("p g i j -> p g (i j)")
                diag = Aflat[:, :, 0:N_F * N_F:N_F + 1]
                if metrics:
                    amax = lup.tile([P, g_sz], F32, tag="amax")
                    nc.vector.tensor_reduce(amax[:, :g_n], Aflat,
                                            mybir.AxisListType.X, Alu.max,
                                            apply_absolute_value=True)
                    minpiv = lup.tile([P, g_sz], F32, tag="minpiv")
                    nc.vector.tensor_reduce(minpiv[:, :g_n], diag,
                                            mybir.AxisListType.X, Alu.min,
                                            apply_absolute_value=True)
                absd = lup.tile([P, g_sz, N_F], F32, tag="absd")
                nc.scalar.activation(absd[:, :g_n], diag, Act.Abs)
                lnd = lup.tile([P, g_sz, N_F], F32, tag="lnd")
                nc.scalar.activation(lnd[:, :g_n], absd[:, :g_n], Act.Ln)
                logabs = lup.tile([P, g_sz], F32, tag="logabs")
                nc.vector.tensor_reduce(logabs[:, :g_n], lnd[:, :g_n],
                                        mybir.AxisListType.X, Alu.add)
                sg = lup.tile([P, g_sz, N_F], F32, tag="sg")
                nc.vector.tensor_scalar(out=sg[:, :g_n], in0=diag,
                                        scalar1=0.0, scalar2=-2.0,
                                        op0=Alu.is_lt, op1=Alu.mult)
                nc.vector.tensor_scalar_add(sg[:, :g_n], sg[:, :g_n], 1.0)
                prodsg = lup.tile([P, g_sz], F32, tag="prodsg")
                nc.vector.tensor_reduce(prodsg[:, :g_n], sg[:, :g_n],
                                        mybir.AxisListType.X, Alu.mult)
                out_t = lup.tile([P, g_sz, n_out], F32, tag="out_t")
                nc.vector.tensor_tensor(out_t[:, :g_n, 0], logabs[:, :g_n],
                                        Jg[:, :g_n], Alu.add)
                nc.vector.tensor_scalar(out=out_t[:, :g_n, 1],
                                        in0=prodsg[:, :g_n],
                                        scalar1=0.0, scalar2=float(np.pi),
                                        op0=Alu.is_lt, op1=Alu.mult)
                if metrics:
                    nc.vector.tensor_copy(out_t[:, :g_n, 2], minpiv[:, :g_n])
                    nc.vector.tensor_copy(out_t[:, :g_n, 3], amax[:, :g_n])
                    nc.vector.tensor_copy(out_t[:, :g_n, 4], rmax2[:, :g_n])
                    nc.vector.tensor_copy(out_t[:, :g_n, 5], csum2[:, :g_n])
                od = out_d[g_lo * P:(g_lo + g_n) * P, :]
                od_pgc = bass.AP(od.tensor, od.offset,
                                 [[n_out, P], [n_out * P, g_n], [1, n_out]])
                nc.sync.dma_start(od_pgc, out_t[:, :g_n])
                g_lo += g_n

    nc.compile()
    return nc


def _get_lu():
    global _cached_lu
    if _cached_lu is None:
        _cached_lu = _build_lu_kernel(N_TILES, G)
    return _cached_lu


def _shared_inputs(M, W, b):
    return {
        "Mmat": np.ascontiguousarray(M), "Wmat": np.ascontiguousarray(W),
        "bvec": np.ascontiguousarray(b[None, :].repeat(P, 0)),
        "wkey": np.ascontiguousarray(
            (N_ORB - np.arange(N_ORB, dtype=np.float32))[None, :].repeat(P, 0)),
    }


def kernel(n, M, W, b, _trace=False):
    n = np.ascontiguousarray(np.asarray(n, dtype=np.float32))
    M = np.ascontiguousarray(np.asarray(M, dtype=np.float32))
    W = np.ascontiguousarray(np.asarray(W, dtype=np.float32))
    b = np.asarray(b, dtype=np.float32)
    assert n.shape == (B, N_ORB) and M.shape == (N_ORB, N_F)

    shared = _shared_inputs(M, W, b)
    S = B // N_CORES

    # ---- pass 1: fp32 unpivoted LU over everything
    nc1 = _get_lu()
    in_maps = [dict(shared, n_shard=np.ascontiguousarray(n[c * S:(c + 1) * S]))
               for c in range(N_CORES)]
    res = run_bass_kernel_spmd(nc1, in_maps, core_ids=list(range(N_CORES)),
                               trace=_trace)
    out = np.empty((B,), np.complex64)
    minpiv = np.empty(B, np.float32)
    amax = np.empty(B, np.float32)
    rmax2 = np.empty(B, np.float32)
    csum2 = np.empty(B, np.float32)
    for c in range(N_CORES):
        o6 = res.results[c]["out6"]
        out[c * S:(c + 1) * S] = o6[:, 0] + 1j * o6[:, 1]
        minpiv[c * S:(c + 1) * S] = o6[:, 2]
        amax[c * S:(c + 1) * S] = o6[:, 3]
        rmax2[c * S:(c + 1) * S] = o6[:, 4]
        csum2[c * S:(c + 1) * S] = o6[:, 5]

    # ---- host-side flagging of numerically-risky samples
    with np.errstate(invalid="ignore"):
        bad = (~np.isfinite(out.real)) | (~np.isfinite(rmax2)) \
            | (~np.isfinite(csum2)) | (rmax2 > FLAG_RMAX2) \
            | (minpiv < FLAG_MINPIV) | (amax > FLAG_AMAX) \
            | (csum2 > FLAG_LMAX)
    flagged = np.nonzero(bad)[0]
    kernel._last_flagged = len(flagged)
    kernel._last_metrics = (minpiv, amax, rmax2, csum2)

    # ---- pass 2: repair flagged samples on host with the same fp32
    # LAPACK slogdet the reference uses (pivoted, so numerically robust).
    if len(flagged) > 0:
        nn = n[flagged]
        idxs = np.argsort(-nn, axis=1, kind="stable")[:, :N_F]
        idxs.sort(axis=1)
        sign, logabs = np.linalg.slogdet(M[idxs])
        Jsel = np.tanh(nn @ W + b[None, :]).sum(axis=1)
        out[flagged] = (logabs + Jsel).astype(np.float32) \
            + 1j * np.where(sign < 0, np.float32(np.pi), np.float32(0.0))

    if _trace:
        kernel._last_results = res
    return out
